# revision 29
# baseline (speedup 1.0000x reference)
"""GAT (3-layer, 2-branch) Bass/Trainium2 kernel for nn_GAT_6854767804552.

Self-contained: hardcodes shapes/sharding. kernel(**inputs) -> (o1, o2).

v2: balanced-window gather plan (6% pad), bf16 tables for 128-wide layers,
merged gather groups (amortize SWDGE fixed cost), branch ping-pong to hide
AllGathers.
"""
import math
import numpy as np
import ml_dtypes

import concourse.bass as bass
import concourse.mybir as mybir
import concourse.tile as tile
from concourse import bacc
from contextlib import ExitStack
from concourse.bass_utils import run_bass_kernel_spmd
from concourse.masks import make_identity

F32 = mybir.dt.float32
BF16 = mybir.dt.bfloat16
I16 = mybir.dt.int16
AF = mybir.ActivationFunctionType
OP = mybir.AluOpType

P = 128
R = 8
N_NODES = 50000
N_GRAPHS = 2048
GPC = N_GRAPHS // R  # 256
NEG = 0.2
DIMS = [(7, 128), (128, 128), (128, 64)]  # (din, dout) per layer
WROWS = 32768  # int16 gather-index window


# ----------------------------------------------------------------- host planning

def _wrap16(flat):
    """int32 flat idx stream -> [128, len/16] int16 wrapped+replicated."""
    flat = np.asarray(flat, dtype=np.int64)
    assert flat.max() <= 32767 and flat.min() >= 0, (flat.min(), flat.max())
    n = len(flat)
    assert n % 16 == 0
    blk = flat.reshape(-1, 16).T.astype(np.int16)
    return np.tile(blk, (8, 1))


def _householder_q(a):
    """Orthogonal-ish Q with last column exactly a; returns (Q, Qinv)."""
    D = len(a)
    na = np.linalg.norm(a)
    u0 = a / na
    e = np.zeros(D); e[-1] = 1.0
    v = e - u0
    nv = np.linalg.norm(v)
    if nv < 1e-7:
        H = np.eye(D)
    else:
        v = v / nv
        H = np.eye(D) - 2.0 * np.outer(v, v)
    Q = H.copy()
    Q[:, -1] = a  # scale last col to a (H[:, -1] == u0)
    S = np.ones(D); S[-1] = 1.0 / na
    Qinv = (S[:, None] * H.T)  # diag(1..1,1/na) @ H^T
    return Q.astype(np.float64), Qinv.astype(np.float64)


def _plan_branch(edge_index, bounds, own, NPAD, WBASE):
    """Balanced-window plan: per-dst A/B split with iterated max(A,B) sort."""
    NB = NPAD // P
    NALL = R * NPAD
    src = np.concatenate([edge_index[0], np.arange(N_NODES, dtype=np.int64)])
    dst = np.concatenate([edge_index[1], np.arange(N_NODES, dtype=np.int64)])
    deg = np.bincount(dst, minlength=N_NODES)
    tgtA = (deg + 1) // 2

    key = -deg.astype(np.int64)
    for it in range(3):
        pos_of = np.zeros(N_NODES, dtype=np.int64)
        for r in range(R):
            ids = np.arange(bounds[r], bounds[r + 1])
            order = ids[np.argsort(key[ids], kind="stable")]
            pos_of[order] = np.arange(len(order))
        row = own * NPAD + pos_of
        erow = row[src]
        cls = np.where(erow < WBASE, 0, np.where(erow >= WROWS, 2, 1))
        a_f = np.bincount(dst[cls == 0], minlength=N_NODES)
        m = np.bincount(dst[cls == 1], minlength=N_NODES)
        x = np.clip(tgtA - a_f, 0, m)
        nA = a_f + x
        nB = deg - nA
        if it < 2:  # keep final iteration's key/assignment consistent
            key = -(np.maximum(nA, nB) * 4096 + np.minimum(nA, nB))

    # final edge-level half assignment
    freeidx = np.nonzero(cls == 1)[0]
    fd = dst[freeidx]
    o = np.argsort(fd, kind="stable")
    sorted_fd = fd[o]
    starts = np.searchsorted(sorted_fd, np.arange(N_NODES))
    rank = np.arange(len(sorted_fd)) - starts[sorted_fd]
    half = np.where(cls == 0, 0, 1)
    half[freeidx[o[rank < x[sorted_fd]]]] = 0

    node_at = np.full((R, NPAD), -1, dtype=np.int64)
    CA = np.zeros(NB, dtype=np.int64)
    CB = np.zeros(NB, dtype=np.int64)
    for r in range(R):
        ids = np.arange(bounds[r], bounds[r + 1])
        order = ids[np.argsort(key[ids], kind="stable")]
        node_at[r, :len(order)] = order
        pad = NPAD - len(order)
        dA = np.concatenate([nA[order], np.zeros(pad, np.int64)]).reshape(NB, P)
        dB = np.concatenate([nB[order], np.zeros(pad, np.int64)]).reshape(NB, P)
        CA = np.maximum(CA, dA.max(axis=1))
        CB = np.maximum(CB, dB.max(axis=1))

    PAD_A = NPAD - 1            # core0's last canonical position (empty)
    PAD_B = NALL - 1 - WBASE    # core7's last position, window-relative

    ia_list, ib_list = [], []
    e_own = own[dst]
    for r in range(R):
        iaparts, ibparts = [], []
        for h, cap, pad in ((0, CA, PAD_A), (1, CB, PAD_B)):
            msk = (e_own == r) & (half == h)
            es, ed = src[msk], dst[msk]
            j = pos_of[ed]
            o2 = np.argsort(j, kind="stable")
            es, j = es[o2], j[o2]
            starts2 = np.searchsorted(j, np.arange(NPAD))
            c = np.arange(len(j)) - starts2[j]
            blk = j // P
            part = j % P
            val = row[es] if h == 0 else row[es] - WBASE
            assert val.min() >= 0 and (len(val) == 0 or val.max() < WROWS)
            for b in range(NB):
                nb = int(cap[b])
                if nb == 0:
                    continue
                arr = np.full((nb, P), pad, dtype=np.int64)
                mb = blk == b
                arr[c[mb], part[mb]] = val[mb]
                (iaparts if h == 0 else ibparts).append(arr.ravel())
        ia_list.append(np.concatenate(iaparts) if iaparts else np.zeros(0, np.int64))
        ib_list.append(np.concatenate(ibparts) if ibparts else np.zeros(0, np.int64))

    return dict(pos_of=pos_of, node_at=node_at, CA=CA, CB=CB,
                ia=ia_list, ib=ib_list)


def _plan(inputs):
    batch = np.asarray(inputs["batch"], dtype=np.int64)
    bounds = np.searchsorted(batch, np.arange(R + 1) * GPC)
    L = np.diff(bounds)
    own = np.repeat(np.arange(R), L)
    NB = math.ceil((L.max() + 1) / P)
    NPAD = NB * P
    WBASE = R * NPAD - WROWS
    assert 0 < WBASE <= WROWS, (NPAD, WBASE)

    b1 = _plan_branch(np.asarray(inputs["edge_index1"], np.int64), bounds, own, NPAD, WBASE)
    b2 = _plan_branch(np.asarray(inputs["edge_index2"], np.int64), bounds, own, NPAD, WBASE)

    # pooling (graph sizes shared across branches)
    sizes = np.bincount(batch, minlength=N_GRAPHS)
    gb_bounds = np.concatenate([[0], np.cumsum(sizes)])
    NGB = GPC // P  # 2
    gorder = np.zeros((R, GPC), dtype=np.int64)
    PC = np.zeros(NGB, dtype=np.int64)
    for r in range(R):
        gl = np.arange(r * GPC, (r + 1) * GPC)
        go = gl[np.argsort(-sizes[gl], kind="stable")]
        gorder[r] = go
        PC = np.maximum(PC, sizes[go].reshape(NGB, P).max(axis=1))

    def pool_stream(plan):
        out = []
        for r in range(R):
            parts = []
            for gb in range(NGB):
                nb = int(PC[gb])
                arr = np.full((nb, P), NPAD, dtype=np.int64)  # pad -> zero row
                for p in range(P):
                    g = gorder[r, gb * P + p]
                    mem = np.arange(gb_bounds[g], gb_bounds[g + 1])
                    arr[:len(mem), p] = plan["pos_of"][mem]
                parts.append(arr.ravel())
            out.append(np.concatenate(parts))
        return out

    return dict(bounds=bounds, L=L, own=own, NB=NB, NPAD=NPAD, WBASE=WBASE,
                b1=b1, b2=b2, sizes=sizes, gorder=gorder, PC=PC,
                ip1=pool_stream(b1), ip2=pool_stream(b2))


def _weights_fold(inputs):
    """Fold rotations into weights. Returns per-layer dicts."""
    out = []
    for l in range(1, 4):
        W = np.asarray(inputs[f"W{l}"], np.float64)
        a_s = np.asarray(inputs[f"as{l}"], np.float64)
        a_d = np.asarray(inputs[f"ad{l}"], np.float64)
        b = np.asarray(inputs[f"b{l}"], np.float64)
        Q, Qinv = _householder_q(a_s)
        Wr = W @ Q
        Waug = np.concatenate([Wr, (W @ a_d)[:, None]], axis=1)
        out.append(dict(Waug=Waug.astype(np.float32),
                        Qinv=Qinv.astype(np.float32),
                        bcol=b.astype(np.float32)[:, None]))
    return out


def _groups(CA, CB, gmax=96, hmax=56):
    """Greedy consecutive-block grouping for merged gathers.

    Returns list of (b0, b1, aoff, boff, acols, bcols) with
    sum(CA[b0:b1]) <= hmax etc (single blocks may exceed)."""
    NB = len(CA)
    out = []
    b0 = 0
    aoff = boff = 0
    while b0 < NB:
        b1 = b0
        ca = cb = 0
        while b1 < NB:
            na, nbb = ca + int(CA[b1]), cb + int(CB[b1])
            if b1 > b0 and (na + nbb > gmax or na > hmax or nbb > hmax):
                break
            ca, cb = na, nbb
            b1 += 1
        out.append((b0, b1, aoff, boff, ca, cb))
        aoff += ca
        boff += cb
        b0 = b1
    return out


# ----------------------------------------------------------------- device build

def _build(meta):
    import os
    MAXL = int(os.environ.get("GAT_MAXL", "3"))
    NBR = int(os.environ.get("GAT_BR", "2"))
    # 8 cols = 1024 descriptors per gather — the SWDGE descriptor carveout
    # holds 1024 (dynamic_dma_scratch_size // 16); larger chunks wedge ucode.
    GSPLIT = int(os.environ.get("GAT_GSPLIT", "8"))
    COPYB = os.environ.get("GAT_COPYB") == "1"       # copy window B table

    NB, NPAD, WBASE = meta["NB"], meta["NPAD"], meta["WBASE"]
    NALL = R * NPAD
    CAb = {1: meta["CA1"], 2: meta["CA2"]}
    CBb = {1: meta["CB1"], 2: meta["CB2"]}
    PC = meta["PC"]
    NGB = len(PC)
    KA = {br: int(sum(CAb[br])) for br in (1, 2)}
    KB = {br: int(sum(CBb[br])) for br in (1, 2)}
    PK = int(sum(PC))
    grp = {br: _groups(CAb[br], CBb[br]) for br in (1, 2)}

    DMASCRATCH = int(os.environ.get("GAT_DMASCRATCH", "16384"))
    nc = bacc.Bacc("TRN2", target_bir_lowering=False, num_swdge_queues=4,
                   dynamic_dma_scratch_size=DMASCRATCH)
    qc = [0]

    def gq():
        qc[0] += 1
        return qc[0] % 4

    # ---------------- inputs
    def din(name, shape, dt=F32):
        return nc.dram_tensor(name, list(shape), dt, kind="ExternalInput")

    xT_in = {1: din("x1T", (7, NPAD)), 2: din("x2T", (7, NPAD))}
    ia_in = {1: din("ia1", (P, KA[1] * 8), I16), 2: din("ia2", (P, KA[2] * 8), I16)}
    ib_in = {1: din("ib1", (P, max(KB[1], 1) * 8), I16),
             2: din("ib2", (P, max(KB[2], 1) * 8), I16)}
    ip_in = {1: din("ip1", (P, PK * 8), I16), 2: din("ip2", (P, PK * 8), I16)}
    xn_in = {1: din("xn1T", (16, GPC)), 2: din("xn2T", (16, GPC))}
    invc_in = din("invc", (P, NGB))
    Wa_in = [din(f"Wa{l}", (DIMS[l - 1][0], DIMS[l - 1][1] + 1)) for l in (1, 2, 3)]
    Qi_in = [din(f"Qi{l}", (DIMS[l - 1][1], DIMS[l - 1][1])) for l in (1, 2, 3)]
    bc_in = [din(f"bc{l}", (DIMS[l - 1][1], 1)) for l in (1, 2, 3)]
    linW_in = din("linW", (80, 64))
    linb_in = din("linb", (P, 64))
    padh_in = din("padh", (1, 128), BF16)   # bf16 table pad row (-1e9 logit)
    pad64_in = din("pad64", (1, 64))        # f32 L3 table pad row
    o_out = {1: nc.dram_tensor("o1", [GPC, 64], F32, kind="ExternalOutput"),
             2: nc.dram_tensor("o2", [GPC, 64], F32, kind="ExternalOutput")}

    with tile.TileContext(nc) as tc, ExitStack() as ctx:
        cst = ctx.enter_context(tc.tile_pool(name="cst", bufs=1))
        sb = ctx.enter_context(tc.tile_pool(name="sb", bufs=2))
        gpool = ctx.enter_context(tc.tile_pool(name="gp", bufs=4))
        ipool = ctx.enter_context(tc.tile_pool(name="ip", bufs=3))
        ps = ctx.enter_context(tc.tile_pool(name="ps", bufs=2, space="PSUM"))
        dr = ctx.enter_context(tc.tile_pool(name="dr", bufs=1, space="DRAM"))

        ident = cst.tile([P, P], F32)
        make_identity(nc, ident[:])
        Wa_sb, Qi_sb, bc_sb = [], [], []
        for l in range(3):
            w = cst.tile([DIMS[l][0], DIMS[l][1] + 1], F32, name=f"wa{l}")
            nc.sync.dma_start(out=w[:], in_=Wa_in[l][:])
            Wa_sb.append(w)
            q = cst.tile([DIMS[l][1], DIMS[l][1]], F32, name=f"qi{l}")
            nc.sync.dma_start(out=q[:], in_=Qi_in[l][:])
            Qi_sb.append(q)
            b = cst.tile([DIMS[l][1], 1], F32, name=f"bcl{l}")
            nc.sync.dma_start(out=b[:], in_=bc_in[l][:])
            bc_sb.append(b)
        linW_sb = cst.tile([80, 64], F32)
        nc.sync.dma_start(out=linW_sb[:], in_=linW_in[:])
        linb_sb = cst.tile([P, 64], F32)
        nc.sync.dma_start(out=linb_sb[:], in_=linb_in[:])
        invc_sb = cst.tile([P, NGB], F32)
        nc.sync.dma_start(out=invc_sb[:], in_=invc_in[:])
        padh_sb = cst.tile([1, 128], BF16, name="padh")
        nc.sync.dma_start(out=padh_sb[:], in_=padh_in[:])
        pad64_sb = cst.tile([1, 64], F32, name="pad64")
        nc.sync.dma_start(out=pad64_sb[:], in_=pad64_in[:])

        # per-branch persistent state
        st = {br: {} for br in (1, 2)}

        def build_layer1(br):
            """x @ Waug1 -> bf16 table rows into ag DRAM + ald."""
            s = st[br]
            xT = sb.tile([7, NPAD], F32, tag=f"xT{br}", bufs=1)
            nc.sync.dma_start(out=xT[:], in_=xT_in[br][:])
            ag = dr.tile([NPAD, 128], BF16, tag=f"ag{br}")
            ald = sb.tile([P, NB], F32, tag=f"ald{br}", bufs=2)
            for b in range(NB):
                ps1 = ps.tile([P, 136], F32, tag="psA")
                nc.tensor.matmul(ps1[:, :129], xT[:, b * P:(b + 1) * P],
                                 Wa_sb[0][:], start=True, stop=True)
                sb1 = sb.tile([P, 128], BF16, tag="sb1h")
                nc.scalar.copy(out=sb1[:], in_=ps1[:, :128])
                nc.vector.tensor_copy(out=ald[:, b:b + 1], in_=ps1[:, 128:129])
                nc.sync.dma_start(out=ag[b * P:(b + 1) * P, :], in_=sb1[:])
            nc.sync.dma_start(out=ag[NPAD - 1:NPAD, :], in_=padh_sb[:])
            s["ag"] = ag
            s["ald"] = ald

        def allgather(br, l):
            s = st[br]
            if l < 3:
                tbl = dr.tile([NALL, 128], BF16, tag=f"tbl{br}h", addr_space="Shared")
            else:
                tbl = dr.tile([NALL, 64], F32, tag=f"tbl{br}f", addr_space="Shared")
            nc.gpsimd.collective_compute(
                "AllGather", OP.bypass, replica_groups=[list(range(R))],
                ins=[s["ag"][:]], outs=[tbl[:]])
            s["tbl"] = tbl
            if COPYB:
                D = 128 if l < 3 else 64
                tblh = dr.tile([WROWS, D], BF16 if l < 3 else F32,
                               tag=f"tblh{br}" + ("h" if l < 3 else "f"))
                nc.sync.dma_start(out=tblh[:], in_=tbl[WBASE:, :])
                s["tblh"] = tblh

        def blocks(br, l, inject=None):
            s = st[br]
            D = DIMS[l - 1][1]
            tbl = s["tbl"]
            ald = s["ald"]
            CA, CB = CAb[br], CBb[br]
            if l < MAXL:
                Dn = DIMS[l][1]
                if l + 1 < 3:
                    ag_next = dr.tile([NPAD, 128], BF16, tag=f"ag{br}")
                else:
                    ag_next = dr.tile([NPAD, 64], F32, tag=f"ag{br}64")
                ald_next = sb.tile([P, NB], F32, tag=f"ald{br}", bufs=2)
            elif l == 3:
                tbl3p = dr.tile([NPAD + 1, 64], F32, tag=f"t3p{br}")
                z64 = sb.tile([1, 64], F32, tag="z64")
                nc.vector.memset(z64[:], 0.0)
                nc.sync.dma_start(out=tbl3p[NPAD:NPAD + 1, :], in_=z64[:])
                s["tbl3p"] = tbl3p

            inj_at = min(3, len(grp[br]) - 1)  # match the 4-deep G ring
            for gi, (b0, b1, aoff, boff, acols, bcols) in enumerate(grp[br]):
                C_grp = acols + bcols
                G = gpool.tile([P, max(C_grp, 1), D], BF16 if l < 3 else F32,
                               tag="G")
                def gat_split(gout_off, ncols, src_ap, itile):
                    step = ncols if GSPLIT <= 0 else GSPLIT
                    for c0 in range(0, ncols, step):
                        cn = min(step, ncols - c0)
                        nc.gpsimd.dma_gather(
                            out_ap=G[:, gout_off + c0:gout_off + c0 + cn, :],
                            in_ap=src_ap,
                            idxs_ap=itile[:, c0 * 8:(c0 + cn) * 8],
                            num_idxs=cn * P, num_idxs_reg=cn * P,
                            elem_size=D, queue_num=gq())

                if acols > 0:
                    iat = ipool.tile([P, max(acols, 1) * 8], I16, tag="iat")
                    nc.sync.dma_start(out=iat[:, :acols * 8],
                                      in_=ia_in[br][:, aoff * 8:(aoff + acols) * 8])
                    gat_split(0, acols, tbl[:], iat)
                if bcols > 0:
                    ibt = ipool.tile([P, max(bcols, 1) * 8], I16, tag="ibt")
                    nc.sync.dma_start(out=ibt[:, :bcols * 8],
                                      in_=ib_in[br][:, boff * 8:(boff + bcols) * 8])
                    bsrc = s["tblh"][:] if COPYB else tbl[WBASE:, :]
                    gat_split(acols, bcols, bsrc, ibt)

                offs = []
                ga, gb_ = aoff, boff
                for b in range(b0, b1):
                    ca, cb = int(CA[b]), int(CB[b])
                    offs.append((b, ca, cb, ga - aoff, acols + (gb_ - boff)))
                    ga += ca
                    gb_ += cb

                # pass 1: logits -> leaky -> exp/den for every block in the
                # group, so the ACT exps pipeline ahead of the DVE chains.
                wts, dens = {}, {}
                for (b, ca, cb, a0, b0c) in offs:
                    C = ca + cb
                    den = sb.tile([P, 1], F32, tag="den", bufs=8)
                    if C == 0:
                        nc.vector.memset(den[:], 0.0)
                        wts[b] = None
                    else:
                        e0 = sb.tile([P, max(C, 1)], F32, tag="e0", bufs=3)
                        if ca > 0:
                            nc.vector.tensor_scalar_add(
                                e0[:, :ca], G[:, a0:a0 + ca, D - 1],
                                ald[:, b:b + 1])
                        if cb > 0:
                            nc.vector.tensor_scalar_add(
                                e0[:, ca:C], G[:, b0c:b0c + cb, D - 1],
                                ald[:, b:b + 1])
                        e2 = sb.tile([P, max(C, 1)], F32, tag="e2", bufs=3)
                        nc.vector.tensor_scalar(e2[:, :C], e0[:, :C], 0.0, NEG,
                                                op0=OP.min, op1=OP.mult)
                        nc.vector.scalar_tensor_tensor(
                            out=e0[:, :C], in0=e0[:, :C], scalar=0.0,
                            in1=e2[:, :C], op0=OP.max, op1=OP.add)
                        w_t = sb.tile([P, max(C, 1)], F32, tag="w_t", bufs=8)
                        nc.scalar.activation(w_t[:, :C], e0[:, :C], AF.Exp,
                                             accum_out=den[:, :1])
                        wts[b] = w_t
                    dens[b] = den

                # pass 2: weighted accumulation + normalize + rotate + ELU
                for (b, ca, cb, a0, b0c) in offs:
                    C = ca + cb
                    acc = sb.tile([P, D], F32, tag="acc")
                    den = dens[b]
                    if C == 0:
                        nc.vector.memset(acc[:], 0.0)
                    else:
                        cols = ([(a0 + i) for i in range(ca)]
                                + [(b0c + i) for i in range(cb)])
                        w_t = wts[b]
                        nc.vector.tensor_scalar_mul(acc[:], G[:, cols[0], :D],
                                                    w_t[:, 0:1])
                        for i in range(1, C):
                            nc.vector.scalar_tensor_tensor(
                                out=acc[:], in0=G[:, cols[i], :D],
                                scalar=w_t[:, i:i + 1],
                                in1=acc[:], op0=OP.mult, op1=OP.add)

                    rcp = sb.tile([P, 1], F32, tag="rcp")
                    nc.vector.tensor_scalar_add(rcp[:], den[:], 1e-30)
                    nc.vector.reciprocal(rcp[:], rcp[:])
                    z = sb.tile([P, D], F32, tag="zt")
                    nc.vector.tensor_scalar_mul(z[:], acc[:], rcp[:, 0:1])

                    psT = ps.tile([P, 136], F32, tag="psB")
                    nc.tensor.transpose(psT[:D, :P], z[:], ident[:])
                    zT = sb.tile([D, P], F32, tag="zT")
                    nc.scalar.copy(out=zT[:], in_=psT[:D, :P])
                    psU = ps.tile([P, 136], F32, tag="psC")
                    nc.tensor.matmul(psU[:D, :P], Qi_sb[l - 1][:], zT[:],
                                     start=True, stop=True)
                    # bias + ELU in transposed layout
                    m_t = sb.tile([D, P], F32, tag="m_t")
                    nc.vector.tensor_scalar(m_t[:], psU[:D, :P],
                                            bc_sb[l - 1][:, 0:1],
                                            0.0, op0=OP.add, op1=OP.min)
                    r_t = sb.tile([D, P], F32, tag="r_t")
                    nc.vector.tensor_scalar(r_t[:], psU[:D, :P],
                                            bc_sb[l - 1][:, 0:1],
                                            0.0, op0=OP.add, op1=OP.max)
                    u_t = sb.tile([D, P], F32, tag="u_t")
                    nc.scalar.activation(u_t[:], m_t[:], AF.Exp)
                    xT_new = sb.tile([D, P], F32, tag="xTn")
                    nc.vector.scalar_tensor_tensor(
                        out=xT_new[:], in0=u_t[:], scalar=-1.0, in1=r_t[:],
                        op0=OP.add, op1=OP.add)

                    if l < MAXL:
                        ps2 = ps.tile([P, 136], F32, tag="psA")
                        nc.tensor.matmul(ps2[:, :Dn + 1], xT_new[:], Wa_sb[l][:],
                                         start=True, stop=True)
                        sb2 = sb.tile([P, Dn], BF16 if l + 1 < 3 else F32,
                                      tag="sb2h" if l + 1 < 3 else "sb2f")
                        nc.scalar.copy(out=sb2[:], in_=ps2[:, :Dn])
                        nc.vector.tensor_copy(out=ald_next[:, b:b + 1],
                                              in_=ps2[:, Dn:Dn + 1])
                        nc.sync.dma_start(out=ag_next[b * P:(b + 1) * P, :],
                                          in_=sb2[:])
                    elif l == 3:
                        psV = ps.tile([P, 136], F32, tag="psB")
                        nc.tensor.transpose(psV[:P, :64], xT_new[:],
                                            ident[:64, :64])
                        sb4 = sb.tile([P, 64], F32, tag="sb4")
                        nc.scalar.copy(out=sb4[:], in_=psV[:P, :64])
                        nc.sync.dma_start(out=tbl3p[b * P:(b + 1) * P, :],
                                          in_=sb4[:])

                # inject the other branch's AllGather here so the Pool
                # engine enters the (blocking) collective with a few groups
                # of gathers already buffered for the compute engines.
                if gi == inj_at and inject is not None:
                    inject()
                    inject = None

            if l < MAXL:
                if l + 1 < 3:
                    nc.sync.dma_start(out=ag_next[NPAD - 1:NPAD, :],
                                      in_=padh_sb[:])
                else:
                    nc.sync.dma_start(out=ag_next[NPAD - 1:NPAD, :],
                                      in_=pad64_sb[:])
                s["ag"] = ag_next
                s["ald"] = ald_next

        def pool_out(br):
            s = st[br]
            if MAXL < 3:
                z0 = sb.tile([P, 64], F32, tag="o_sb")
                nc.vector.memset(z0[:], 0.0)
                for gb in range(NGB):
                    nc.sync.dma_start(out=o_out[br][gb * P:(gb + 1) * P, :],
                                      in_=z0[:])
                return
            tbl3p = s["tbl3p"]
            xnT = sb.tile([16, GPC], F32, tag="xnT")
            nc.sync.dma_start(out=xnT[:], in_=xn_in[br][:])
            offP = 0
            for gb in range(NGB):
                pc = int(PC[gb])
                Gp = gpool.tile([P, max(pc, 1), 64], F32, tag="Gp", bufs=2)
                ipt = ipool.tile([P, max(pc, 1) * 8], I16, tag="ipt")
                nc.sync.dma_start(out=ipt[:, :pc * 8],
                                  in_=ip_in[br][:, offP * 8:(offP + pc) * 8])
                step = pc if GSPLIT <= 0 else GSPLIT
                for c0 in range(0, pc, step):
                    cn = min(step, pc - c0)
                    nc.gpsimd.dma_gather(
                        out_ap=Gp[:, c0:c0 + cn, :], in_ap=tbl3p[:],
                        idxs_ap=ipt[:, c0 * 8:(c0 + cn) * 8],
                        num_idxs=cn * P, num_idxs_reg=cn * P,
                        elem_size=64, queue_num=gq())
                offP += pc

                accp = sb.tile([P, 64], F32, tag="accp")
                nc.vector.tensor_copy(out=accp[:], in_=Gp[:, 0, :])
                for c in range(1, pc):
                    nc.vector.tensor_tensor(out=accp[:], in0=accp[:],
                                            in1=Gp[:, c, :], op=OP.add)
                nc.vector.tensor_scalar_mul(accp[:], accp[:],
                                            invc_sb[:, gb:gb + 1])

                psP = ps.tile([P, 136], F32, tag="psB")
                nc.tensor.transpose(psP[:64, :P], accp[:], ident[:])
                lhsT = sb.tile([80, P], F32, tag="lhsT")
                nc.scalar.copy(out=lhsT[:64, :], in_=psP[:64, :P])
                nc.sync.dma_start(out=lhsT[64:80, :],
                                  in_=xnT[:, gb * P:(gb + 1) * P])
                psO = ps.tile([P, 136], F32, tag="psC")
                nc.tensor.matmul(psO[:, :64], lhsT[:], linW_sb[:],
                                 start=True, stop=True)
                o_sb = sb.tile([P, 64], F32, tag="o_sb")
                nc.vector.tensor_tensor(out=o_sb[:], in0=psO[:, :64],
                                        in1=linb_sb[:], op=OP.add)
                nc.sync.dma_start(out=o_out[br][gb * P:(gb + 1) * P, :],
                                  in_=o_sb[:])

        # ---- schedule: branch ping-pong; each AllGather is injected into
        # the middle of the OTHER branch's block loop (the collective blocks
        # the Pool engine, so gathers must be queued ahead of it).
        brs = (1, 2)[:NBR]
        if NBR == 2 and MAXL >= 1:
            # AG(1,1) transfers while branch 2's build matmuls run
            build_layer1(1)
            allgather(1, 1)
            build_layer1(2)
        else:
            for br in brs:
                build_layer1(br)
        if MAXL >= 1 and NBR == 1:
            allgather(1, 1)
            for l in range(1, MAXL + 1):
                blocks(1, l)
                if l < MAXL:
                    allgather(1, l + 1)
            pool_out(1)
        elif MAXL >= 1 and NBR == 2:
            for l in range(1, MAXL + 1):
                blocks(1, l, inject=lambda ll=l: allgather(2, ll))
                if l == MAXL:
                    pool_out(1)
                    blocks(2, l)
                else:
                    blocks(2, l, inject=lambda ll=l + 1: allgather(1, ll))
            pool_out(2)
        for br in brs:
            if MAXL < 1:
                pool_out(br)
        if NBR == 0:
            for br in (1, 2):
                z0 = sb.tile([P, 64], F32, tag="o_sb")
                nc.vector.memset(z0[:], 0.0)
                for gb in range(NGB):
                    nc.sync.dma_start(out=o_out[br][gb * P:(gb + 1) * P, :],
                                      in_=z0[:])

    nc.compile()
    return nc


# ----------------------------------------------------------------- entry point

_CACHE = {}
LAST_RES = None
LAST_RUN_S = None


def _get_nc(meta):
    key = (meta["NB"], meta["WBASE"],
           tuple(meta["CA1"]), tuple(meta["CB1"]),
           tuple(meta["CA2"]), tuple(meta["CB2"]), tuple(meta["PC"]))
    if key not in _CACHE:
        _CACHE[key] = _build(meta)
    return _CACHE[key]


def _prepare(plan, inputs):
    NB, NPAD = plan["NB"], plan["NPAD"]
    wf = _weights_fold(inputs)

    meta = dict(NB=NB, NPAD=NPAD, WBASE=plan["WBASE"],
                CA1=plan["b1"]["CA"], CB1=plan["b1"]["CB"],
                CA2=plan["b2"]["CA"], CB2=plan["b2"]["CB"],
                PC=plan["PC"])

    gorder = plan["gorder"]
    sizes = plan["sizes"]
    NGB = len(plan["PC"])

    padh = np.zeros((1, 128), ml_dtypes.bfloat16)
    padh[0, -1] = -1e9
    pad64 = np.zeros((1, 64), np.float32)
    pad64[0, -1] = -1e9

    invc_full = 1.0 / np.maximum(sizes, 1.0)

    in_maps = []
    for r in range(R):
        m = {}
        for br, bp in ((1, plan["b1"]), (2, plan["b2"])):
            x = np.asarray(inputs[f"x{br}"], np.float32)
            ids = bp["node_at"][r]
            xT = np.zeros((7, NPAD), np.float32)
            valid = ids >= 0
            xT[:, valid] = x[ids[valid]].T
            m[f"x{br}T"] = xT
            ka = len(bp["ia"][r])
            m[f"ia{br}"] = _wrap16(bp["ia"][r]) if ka else np.zeros((P, 8), np.int16)
            kb = len(bp["ib"][r])
            m[f"ib{br}"] = _wrap16(bp["ib"][r]) if kb else np.zeros((P, 8), np.int16)
            m[f"ip{br}"] = _wrap16(plan[f"ip{br}"][r])
            xn = np.asarray(inputs[f"x_norm2_{br}"], np.float32)
            m[f"xn{br}T"] = np.ascontiguousarray(xn[gorder[r]].T)
        ic = np.zeros((P, NGB), np.float32)
        for gb in range(NGB):
            ic[:, gb] = invc_full[gorder[r, gb * P:(gb + 1) * P]]
        m["invc"] = ic
        for l in (1, 2, 3):
            m[f"Wa{l}"] = wf[l - 1]["Waug"]
            m[f"Qi{l}"] = wf[l - 1]["Qinv"]
            m[f"bc{l}"] = wf[l - 1]["bcol"]
        m["linW"] = np.asarray(inputs["linW"], np.float32)
        m["linb"] = np.tile(np.asarray(inputs["linb"], np.float32)[None, :], (P, 1))
        m["padh"] = padh
        m["pad64"] = pad64
        in_maps.append(m)
    return meta, in_maps


_PLAN_CACHE = {}


def kernel(**inputs):
    import zlib
    fp = tuple(
        (k, np.asarray(inputs[k]).shape,
         zlib.adler32(np.ascontiguousarray(inputs[k]).tobytes()))
        for k in sorted(inputs))
    if fp in _PLAN_CACHE:
        plan, meta, in_maps = _PLAN_CACHE[fp]
    else:
        plan = _plan(inputs)
        meta, in_maps = _prepare(plan, inputs)
        _PLAN_CACHE.clear()
        _PLAN_CACHE[fp] = (plan, meta, in_maps)
    nc = _get_nc(meta)
    gorder = plan["gorder"]

    import os, time as _time
    trace = os.environ.get("GAT_TRACE") == "1"
    _t0 = _time.time()
    res = run_bass_kernel_spmd(nc, in_maps, core_ids=list(range(R)), trace=trace)
    global LAST_RES, LAST_RUN_S
    LAST_RES = res
    LAST_RUN_S = _time.time() - _t0

    o1 = np.zeros((N_GRAPHS, 64), np.float32)
    o2 = np.zeros((N_GRAPHS, 64), np.float32)
    for r in range(R):
        o1[gorder[r]] = res.results[r]["o1"]
        o2[gorder[r]] = res.results[r]["o2"]
    return o1, o2
